# revision 1
# baseline (speedup 1.0000x reference)
"""Distributed causal multi-head attention forward for one TRN2 chip (8 NeuronCores).

Problem (nn_Attention): B=2, S=2048, d_model=1024, 16 heads x 64.
    attn_in = x + pos_embed
    q = attn_in @ W_Q + b_Q ; k = attn_in @ W_K + b_K ; v = x @ W_V + b_V
    out = softmax(causal(q k^T / sqrt(64))) v @ W_O + b_O

Sharding: data-parallel over batch (2 groups of 4 cores), tensor-parallel over
heads inside each group (4 heads per core).  Each core computes the partial
output  sum_h z_h @ W_O_h  for its heads, then a ReduceScatter(add) over the
4-core group leaves each core with S/4 rows of the fully-summed output.  The
host reassembles the full [B, S, D] tensor from the 8 row shards (pure
gather/indexing, no arithmetic).

On-device layout notes:
  * The host passes x/pos transposed ([D, S]) so the d_model contraction axis
    lands on SBUF partitions without on-chip transposes.
  * All matmul operands are float32r (single-pass fp32 on the PE array, ~4x
    faster than float32, ~1.5e-4 relative error).  PSUM accumulation is fp32.
  * Scores are built transposed (keys on partitions) so softmax-exp feeds the
    P@V matmul directly.  The softmax denominator comes from a ones-column
    appended to V; the causal mask is a 0/1 multiply after exp; division by
    the denominator happens via a K=1 broadcast matmul of the reciprocal.
"""

import numpy as np

import concourse.bass as bass  # noqa: F401  (bass must import before bacc)
import concourse.mybir as mybir
from concourse import bacc, tile
from concourse.bass_utils import run_bass_kernel_spmd

B, S, D = 2, 2048, 1024
NH, DH = 16, 64
N_CORES = 8
GPC = 4                      # cores per batch group
HPC = NH // GPC              # heads per core
QB = 512                     # query-block rows
NJ = S // QB                 # query blocks
KCH = 128                    # key chunk (= row tile)
DCH = D // 128               # d_model chunks
RG = [[0, 1, 2, 3], [4, 5, 6, 7]]
SCALE = 1.0 / float(np.sqrt(DH))

F32 = mybir.dt.float32
F32R = mybir.dt.float32r
EXP = mybir.ActivationFunctionType.Exp


def build_nc(reps: int = 1, collective: bool = True, ab: str = "",
             fused_exp: bool = True, osb_alt: bool = False,
             psa2: bool = True, bias: bool = True):
    """Build the per-core Bass graph.  `reps` repeats the whole computation
    (used only for wall-clock timing calibration; grading uses reps=1)."""
    nc = bacc.Bacc("TRN2", target_bir_lowering=False, debug=False,
                   num_devices=N_CORES)

    xT = nc.dram_tensor("xT", [D, S], F32R, kind="ExternalInput").ap()
    posT = nc.dram_tensor("posT", [D, S], F32R, kind="ExternalInput").ap()
    wq = nc.dram_tensor("wq", [D, HPC * DH], F32R, kind="ExternalInput").ap()
    wk = nc.dram_tensor("wk", [D, HPC * DH], F32R, kind="ExternalInput").ap()
    wv = nc.dram_tensor("wv", [D, HPC * DH], F32R, kind="ExternalInput").ap()
    wo = nc.dram_tensor("wo", [DH, HPC * D], F32R, kind="ExternalInput").ap()
    bq = nc.dram_tensor("bq", [1, HPC * DH], F32R, kind="ExternalInput").ap()
    bk = nc.dram_tensor("bk", [1, HPC * DH], F32R, kind="ExternalInput").ap()
    bv = nc.dram_tensor("bv", [1, HPC * DH], F32R, kind="ExternalInput").ap()
    bo = nc.dram_tensor("bo", [1, D], F32R, kind="ExternalInput").ap()
    masks = nc.dram_tensor("masks", [KCH, 2 * KCH], F32R,
                           kind="ExternalInput").ap()
    out_ext = nc.dram_tensor("out", [S // GPC, D], F32,
                             kind="ExternalOutput").ap()

    with tile.TileContext(nc) as tc:
        with tc.tile_pool(name="wp", bufs=1) as wp, \
             tc.tile_pool(name="qkv", bufs=1) as qp, \
             tc.tile_pool(name="zh", bufs=1) as zhp, \
             tc.tile_pool(name="xcp", bufs=13) as xcp, \
             tc.tile_pool(name="posp", bufs=3) as posp, \
             tc.tile_pool(name="ptp", bufs=4) as ptp, \
             tc.tile_pool(name="lrp", bufs=2) as lrp, \
             tc.tile_pool(name="osb", bufs=3) as osbp, \
             tc.tile_pool(name="psA", bufs=(2 if psa2 else 1),
                          space="PSUM") as psA, \
             tc.tile_pool(name="psS", bufs=2, space="PSUM") as psS, \
             tc.tile_pool(name="psZ", bufs=2, space="PSUM") as psZ, \
             tc.tile_pool(name="psO", bufs=1, space="PSUM") as psO, \
             tc.tile_pool(name="dram", bufs=2, space="DRAM") as dp:

            # ---------- persistent weights / constants ----------
            wq_c, wk_c, wv_c = [], [], []
            for kc in range(DCH):
                sl = slice(128 * kc, 128 * (kc + 1))
                t_wq = wp.tile([128, HPC * DH], F32R, tag=f"wq{kc}")
                nc.sync.dma_start(t_wq[:], wq[sl, :])
                wq_c.append(t_wq)
                t_wk = wp.tile([128, HPC * DH], F32R, tag=f"wk{kc}")
                nc.sync.dma_start(t_wk[:], wk[sl, :])
                wk_c.append(t_wk)
                t_wv = wp.tile([128, HPC * DH], F32R, tag=f"wv{kc}")
                nc.sync.dma_start(t_wv[:], wv[sl, :])
                wv_c.append(t_wv)
            wo_t = wp.tile([DH, HPC * D], F32R, tag="wo")
            nc.sync.dma_start(wo_t[:], wo[:, :])
            # causal triangle band mask (both head-halves): tri[k, :, j] =
            # 1 if k <= j
            tri_m = wp.tile([KCH, 2, KCH], F32R, tag="tri_m")
            nc.sync.dma_start(tri_m[:, :, :], masks[:, :])
            bq_t = wp.tile([1, HPC * DH], F32R, tag="bq")
            nc.sync.dma_start(bq_t[:], bq[:, :])
            bk_t = wp.tile([1, HPC * DH], F32R, tag="bk")
            nc.sync.dma_start(bk_t[:], bk[:, :])
            bv_t = wp.tile([1, HPC * DH], F32R, tag="bv")
            nc.sync.dma_start(bv_t[:], bv[:, :])
            bo_t = wp.tile([1, D], F32R, tag="bo")
            nc.sync.dma_start(bo_t[:], bo[:, :])
            ones_f = wp.tile([128, QB], F32, tag="ones_f")
            nc.vector.memset(ones_f[:], 1.0)
            ones = wp.tile([128, QB], F32R, tag="ones")
            nc.vector.tensor_copy(ones[:], ones_f[:])

            # persistent per-rep activations
            qT = []
            kT = []
            for p in range(2):
                t_q = qp.tile([128, S], F32R, tag=f"qT{p}")
                qT.append(t_q)
                t_k = qp.tile([128, S], F32R, tag=f"kT{p}")
                kT.append(t_k)
            v_aug = []
            for rt in range(S // KCH):
                t_v = qp.tile([128, HPC, DH + 1], F32R, tag=f"va{rt}")
                nc.vector.tensor_copy(t_v[:, :, DH:DH + 1], ones[:, 0:HPC])
                v_aug.append(t_v)
            zh_t = []
            for h in range(HPC):
                t_z = zhp.tile([DH, QB], F32R, tag=f"zh{h}")
                zh_t.append(t_z)

            for _rep in range(reps):
                for jb in range(NJ):
                    jsl = slice(QB * jb, QB * (jb + 1))
                    # ---------- phase A: projections for this row block ----
                    xc_t, xpc_t = [], []
                    for kc in range(DCH):
                        ksl = slice(128 * kc, 128 * (kc + 1))
                        t_xc = xcp.tile([128, QB], F32R, tag="xc")
                        if ab == "dma2x":
                            nc.sync.dma_start(t_xc[:], xT[ksl, jsl])
                        nc.sync.dma_start(t_xc[:], xT[ksl, jsl])
                        t_pos = posp.tile([128, QB], F32R, tag="pos")
                        if ab == "dma2x":
                            nc.sync.dma_start(t_pos[:], posT[ksl, jsl])
                        nc.sync.dma_start(t_pos[:], posT[ksl, jsl])
                        t_xpc = xcp.tile([128, QB], F32R, tag="xpc")
                        nc.gpsimd.tensor_add(t_xpc[:], t_xc[:], t_pos[:])
                        xc_t.append(t_xc)
                        xpc_t.append(t_xpc)
                    for dst, w_c, b_t in ((qT, wq_c, bq_t), (kT, wk_c, bk_t)):
                        for p in range(2):
                            psl = slice(128 * p, 128 * (p + 1))
                            acc = psA.tile([128, QB], F32, tag="a_ps")
                            for kc in range(DCH):
                                nc.tensor.matmul(
                                    acc[:], w_c[kc][:, psl], xpc_t[kc][:],
                                    start=(kc == 0),
                                    stop=(not bias and kc == DCH - 1))
                            if bias:
                                nc.tensor.matmul(acc[:], b_t[0:1, psl],
                                                 ones[0:1, 0:QB],
                                                 start=False, stop=True)
                            nc.scalar.copy(dst[p][:, jsl], acc[:])
                    for r in range(4):
                        rt = 4 * jb + r
                        rsl = slice(128 * r, 128 * (r + 1))
                        vacc = psA.tile([128, HPC * DH], F32, tag="a_ps")
                        for kc in range(DCH):
                            nc.tensor.matmul(
                                vacc[:], xc_t[kc][:, rsl], wv_c[kc][:],
                                start=(kc == 0),
                                stop=(not bias and kc == DCH - 1))
                        if bias:
                            nc.tensor.matmul(vacc[:], ones[0:1, 0:128],
                                             bv_t[0:1, :],
                                             start=False, stop=True)
                        va = v_aug[rt]
                        nc.vector.tensor_copy(va[:, :, 0:DH], vacc[:])

                    # ---------- phase B: attention for J = jb ---------------
                    J = jb
                    nch = 4 * (J + 1)
                    for hp in range(2):
                        h0, h1 = 2 * hp, 2 * hp + 1
                        lo = slice(0, 64)
                        hi = slice(64, 128)
                        z0 = psZ.tile([DH + 1, QB], F32, tag="z_ps")
                        z1 = psZ.tile([DH + 1, QB], F32, tag="z_ps")
                        for c in range(nch):
                            dlt = c - 4 * J
                            w0 = 128 * dlt if dlt >= 0 else 0  # causal col start
                            csl = slice(KCH * c, KCH * (c + 1))
                            qsl = slice(QB * J + w0, QB * (J + 1))
                            s2 = psS.tile([KCH, 2, QB], F32, tag="s2")
                            nc.tensor.matmul(s2[:, 0, w0:QB],
                                             kT[hp][lo, csl],
                                             qT[hp][lo, qsl],
                                             start=True, stop=True)
                            nc.tensor.matmul(s2[:, 1, w0:QB],
                                             kT[hp][hi, csl], qT[hp][hi, qsl],
                                             start=True, stop=True)
                            p2 = ptp.tile([KCH, 2, QB], F32R, tag="pT")
                            if fused_exp:
                                nc.scalar.activation(p2[:, :, w0:QB],
                                                     s2[:, :, w0:QB], EXP,
                                                     scale=SCALE)
                            else:
                                nc.scalar.activation(p2[:, 0, w0:QB],
                                                     s2[:, 0, w0:QB], EXP,
                                                     scale=SCALE)
                                nc.scalar.activation(p2[:, 1, w0:QB],
                                                     s2[:, 1, w0:QB], EXP,
                                                     scale=SCALE)
                            if dlt >= 0:
                                if fused_exp:
                                    # causal triangle band of each half
                                    nc.vector.tensor_mul(
                                        p2[:, :, w0:w0 + KCH],
                                        p2[:, :, w0:w0 + KCH], tri_m[:])
                                else:
                                    nc.vector.tensor_mul(
                                        p2[:, 0, w0:w0 + KCH],
                                        p2[:, 0, w0:w0 + KCH],
                                        tri_m[:, 0, :])
                                    nc.vector.tensor_mul(
                                        p2[:, 1, w0:w0 + KCH],
                                        p2[:, 1, w0:w0 + KCH],
                                        tri_m[:, 1, :])
                            nc.tensor.matmul(z0[:, w0:QB],
                                             v_aug[c][:, h0, :],
                                             p2[:, 0, w0:QB],
                                             start=(c == 0),
                                             stop=(c == nch - 1))
                            nc.tensor.matmul(z1[:, w0:QB],
                                             v_aug[c][:, h1, :],
                                             p2[:, 1, w0:QB],
                                             start=(c == 0),
                                             stop=(c == nch - 1))
                        # normalize: z / l via reciprocal + K=1 broadcast mm
                        rsb = lrp.tile([DH + 1, 2 * QB], F32R, tag="r_sb")
                        with nc.allow_low_precision(
                                reason="f32r reciprocal feeds f32r matmul"):
                            nc.vector.reciprocal(rsb[DH:DH + 1, 0:QB],
                                                 z0[DH:DH + 1, :])
                            nc.vector.reciprocal(rsb[DH:DH + 1, QB:2 * QB],
                                                 z1[DH:DH + 1, :])
                        r2 = psS.tile([DH, 2, QB], F32, tag="s2")
                        nc.tensor.matmul(r2[:, 0, :], ones[DH:DH + 1, 0:DH],
                                         rsb[DH:DH + 1, 0:QB],
                                         start=True, stop=True)
                        nc.tensor.matmul(r2[:, 1, :],
                                         ones[DH:DH + 1, 0:DH],
                                         rsb[DH:DH + 1, QB:2 * QB],
                                         start=True, stop=True)
                        rbc = lrp.tile([DH, 2, QB], F32, tag="rbc")
                        nc.vector.tensor_copy(rbc[:], r2[:])
                        nc.vector.tensor_mul(zh_t[h0][:], z0[0:DH, :],
                                             rbc[:, 0, :])
                        nc.vector.tensor_mul(zh_t[h1][:], z1[0:DH, :],
                                             rbc[:, 1, :])

                    # ---------- phase C: W_O partial + ReduceScatter --------
                    prt = dp.tile([QB, D], F32, tag="part")
                    for pt_i in range(4):
                        ptsl = slice(128 * pt_i, 128 * (pt_i + 1))
                        for ms in range(2):
                            msl = slice(512 * ms, 512 * (ms + 1))
                            if psa2:
                                oacc = psA.tile([128, 512], F32,
                                                tag="a_ps")
                            else:
                                oacc = psO.tile([128, 512], F32,
                                                tag="o_ps")
                            for h in range(HPC):
                                nc.tensor.matmul(
                                    oacc[:], zh_t[h][:, ptsl],
                                    wo_t[:, D * h + 512 * ms:
                                         D * h + 512 * (ms + 1)],
                                    start=(h == 0),
                                    stop=(not bias and h == HPC - 1))
                            if bias:
                                nc.tensor.matmul(oacc[:], ones[0:1, 0:128],
                                                 bo_t[0:1, msl],
                                                 start=False, stop=True)
                            o_sb = osbp.tile([128, 512], F32, tag="o_sb")
                            if osb_alt and (pt_i + ms) % 2 == 1:
                                nc.scalar.copy(o_sb[:], oacc[:])
                            else:
                                nc.vector.tensor_copy(o_sb[:], oacc[:])
                            nc.sync.dma_start(prt[ptsl, msl], o_sb[:])
                            if ab == "out2x":
                                prt2 = dp.tile([128, 512], F32, tag="prt2")
                                nc.sync.dma_start(prt2[:, :], o_sb[:])
                    if collective:
                        rs = dp.tile([QB // GPC, D], F32, tag="rs")
                        nc.gpsimd.collective_compute(
                            "ReduceScatter", mybir.AluOpType.add,
                            replica_groups=RG,
                            ins=[prt[:].opt()], outs=[rs[:].opt()])
                        nc.sync.dma_start(out_ext[128 * J:128 * (J + 1), :],
                                          rs[:])
                    else:
                        # timing-sim variant: skip the collective
                        nc.sync.dma_start(out_ext[128 * J:128 * (J + 1), :],
                                          prt[0:128, :])
    nc.compile()
    return nc


def _make_masks():
    # [128, 2*128] causal triangle duplicated for the head-pair layout:
    # tri[k, j] = 1 if k <= j (the diagonal band of every 128-key chunk
    # relative to its causal column start)
    k = np.arange(KCH)[:, None]
    j = np.arange(KCH)[None, :]
    tri = (k <= j).astype(np.float32)
    return np.ascontiguousarray(np.concatenate([tri, tri], axis=1))


def make_in_maps(x, pos_embed, W_Q, b_Q, W_K, b_K, W_V, b_V, W_O, b_O):
    x = np.asarray(x, np.float32)
    pos_embed = np.asarray(pos_embed, np.float32)
    W_Q = np.asarray(W_Q, np.float32)
    W_K = np.asarray(W_K, np.float32)
    W_V = np.asarray(W_V, np.float32)
    W_O = np.asarray(W_O, np.float32)
    b_Q = np.asarray(b_Q, np.float32)
    b_K = np.asarray(b_K, np.float32)
    b_V = np.asarray(b_V, np.float32)
    b_O = np.asarray(b_O, np.float32)
    masks = _make_masks()
    in_maps = []
    for c in range(N_CORES):
        g, j = divmod(c, GPC)
        hs = slice(HPC * j, HPC * (j + 1))
        in_maps.append({
            "xT": np.ascontiguousarray(x[g].T),
            "posT": np.ascontiguousarray(pos_embed[g].T),
            "wq": np.ascontiguousarray(
                W_Q[hs].transpose(1, 0, 2).reshape(D, HPC * DH)),
            "wk": np.ascontiguousarray(
                W_K[hs].transpose(1, 0, 2).reshape(D, HPC * DH)),
            "wv": np.ascontiguousarray(
                W_V[hs].transpose(1, 0, 2).reshape(D, HPC * DH)),
            "wo": np.ascontiguousarray(
                W_O[hs].transpose(1, 0, 2).reshape(DH, HPC * D)),
            "bq": np.ascontiguousarray(b_Q[hs].reshape(1, HPC * DH)),
            "bk": np.ascontiguousarray(b_K[hs].reshape(1, HPC * DH)),
            "bv": np.ascontiguousarray(b_V[hs].reshape(1, HPC * DH)),
            "bo": np.ascontiguousarray(b_O.reshape(1, D)),
            "masks": masks,
        })
    return in_maps


def assemble_out(results):
    out = np.empty((B, S, D), np.float32)
    for c in range(N_CORES):
        g, j = divmod(c, GPC)
        o = results[c]["out"].reshape(NJ, 128, D)
        for J in range(NJ):
            out[g, QB * J + 128 * j:QB * J + 128 * (j + 1), :] = o[J]
    return out


_BUILT = {}


def get_built(reps: int = 1, bias: bool = True):
    key = (reps, bias)
    if key not in _BUILT:
        _BUILT[key] = build_nc(reps, bias=bias)
    return _BUILT[key]


def kernel(**inputs) -> np.ndarray:
    use_bias = any(
        np.any(np.asarray(inputs[k])) for k in ("b_Q", "b_K", "b_V", "b_O"))
    nc = get_built(1, bias=bool(use_bias))
    in_maps = make_in_maps(**inputs)
    res = run_bass_kernel_spmd(nc, in_maps, list(range(N_CORES)))
    return assemble_out(res.results)

